# revision 4
# baseline (speedup 1.0000x reference)
"""Expert-parallel grouped MLP (MoE routing) for Trainium2, fp16 + Strassen.

Problem: x[16384,1024] fp32, w1[8,1024,4096], w2[8,4096,1024],
rows_per_expert=2048.  out = gelu(x_e @ w1[e]) @ w2[e] per expert group.

Sharding: one expert per NeuronCore (E=8 == n_cores), no collectives.

Speed strategy vs the bf16 direct baseline (466us):
  * fp16 operands everywhere.  The PE runs fp16 at the same 1 cycle/row as
    bf16 but with 10 mantissa bits instead of 7 (direct rel-err 4e-4 vs
    3.4e-3), buying precision headroom that funds Strassen.
  * One level of Strassen on GEMM1: per 1024-token pair, split
    [1024t,1024h]@[1024h,4096f] into 2x2x2 blocks and compute 7 products
    (4-chunk PSUM chains of 512-free matmuls) instead of 8 -> 12.5% fewer
    GEMM1 matmuls.  The w1 block combos (B11+B22 etc.) are precomputed on
    the host in fp32; the x block combos are 5 cheap fp16 adds on DVE.
    The 7 PSUM products are recombined into the 4 C quadrants with the
    minimum 7 PSUM-reading ops (5 evict-casts + 2 one-PSUM-operand
    tensor_tensors) plus 6 fp16 SBUF adds, split across Scalar/DVE/GpSimd
    so each engine stays under ~70% while the PE streams.
  * GEMM2 stays direct (its Strassen would need 5 inter-combos of 2MB
    each on the device - SBUF and DVE can't pay for it).
  * Simulated end-to-end error of this exact dataflow: ~8e-4 (vs 1e-2
    gate).

Layouts (host pre-permuted so every DMA line is >=1KB contiguous):
    xp  -> [2, 128, 8, 1024]   xp[pr,p,hc,ti]    = x[pr*1024+ti, hc*128+p]
    w1c -> [16, 128, 7*4*128]  w1c[j,p,i,c,fi]   = w1combo_i[c*128+p, j*128+fi]
    w2p -> [8, 128, 4096]      w2p[h,p,f*128+hi] = w2[f*128+p, h*128+hi]
    out4<- [2, 2, 8, 128, 512] out4[pr,th,hc,p,ti] = out[pr*1024+th*512+ti, hc*128+p]
"""

import numpy as np

E = 8
H = 1024
F = 4096
T_PER_E = 2048
P = 128
NPAIR = 2      # token pairs of 1024
TP = 1024      # tokens per pair
TH = 512       # free dim = half pair (Strassen M/2)
NJ = 16        # G1 f-tile groups per N-half
KC = 4         # G1 contraction chunks per product (512/128)
FO = 32        # f chunks of F
NHO = 8        # G2 h-tiles
NWARM = 12     # PE warm-up matmuls

# G1 Strassen chains, in PE emission order.
# (name, w1c product idx, rhs spec); rhs: ("xb", hc0, t0) = plain x block
# view, ("xc", slot) = x combo slot.
# Products (std Strassen): P1=(A11+A22)(B11+B22) P2=(A21+A22)B11
# P3=A11(B12-B22) P4=A22(B21-B11) P5=(A11+A12)B22 P6=(A21-A11)(B11+B12)
# P7=(A12-A22)(B21+B22)
# C11=P1+P4-P5+P7  C12=P3+P5  C21=P2+P4  C22=P1-P2+P3+P6
CHAINS = [
    ("p3", 2, ("xb", 0, 0)),
    ("p4", 3, ("xb", 4, TH)),
    ("p2", 1, ("xc", 1)),
    ("p5", 4, ("xc", 2)),
    ("p1", 0, ("xc", 0)),
    ("p6", 5, ("xc", 3)),
    ("p7", 6, ("xc", 4)),
]
# x combo slots: 0:A11+A22 1:A21+A22 2:A11+A12 3:A21-A11 4:A12-A22
# A11 = xb[:, c, 0:TH]   A12 = xb[:, 4+c, 0:TH]
# A21 = xb[:, c, TH:]    A22 = xb[:, 4+c, TH:]
XC_SPEC = [
    (0, (0, 0), (4, TH), True),
    (1, (0, TH), (4, TH), True),
    (2, (0, 0), (4, 0), True),
    (3, (0, TH), (0, 0), False),
    (4, (4, 0), (4, TH), False),
]

TRACE = False          # test.py sets kernel.TRACE = True for profiling
LAST_RESULTS = None    # BassKernelResults of the most recent run

_nc_cache = None


def _build_nc():
    import concourse.mybir as mybir
    import concourse.tile as tile
    from concourse import bacc
    from concourse.tile_rust import add_dep_helper

    f16 = mybir.dt.float16
    f32 = mybir.dt.float32
    GELU = mybir.ActivationFunctionType.Gelu_apprx_tanh
    ADD = mybir.AluOpType.add
    SUB = mybir.AluOpType.subtract

    nc = bacc.Bacc("TRN2", target_bir_lowering=False, debug=False)

    xp = nc.dram_tensor("xp", [NPAIR, P, 8, TP], f16, kind="ExternalInput").ap()
    w1c = nc.dram_tensor("w1c", [NJ, P, 7 * KC * P], f16, kind="ExternalInput").ap()
    w2p = nc.dram_tensor("w2p", [NHO, P, F], f16, kind="ExternalInput").ap()
    out4 = nc.dram_tensor("out4", [NPAIR, 2, NHO, P, TH], f16,
                          kind="ExternalOutput").ap()

    mm_first = {}   # (pair, j) -> first matmul of G1 group
    g2_first = {}   # (pair, chain_k) -> first matmul of G2 chain
    gated = []      # (dma_instr, gate_key) resolved at the end
    xb_tiles = {}
    xc_tiles = {}

    with tile.TileContext(nc) as tc:
        with (
            tc.tile_pool(name="wpool", bufs=1) as wpool,
            tc.tile_pool(name="w1pool", bufs=3) as w1pool,
            tc.tile_pool(name="w2pool", bufs=3) as w2pool,
            tc.tile_pool(name="xpool", bufs=2) as xpool,
            tc.tile_pool(name="xcpool", bufs=1) as xcpool,
            tc.tile_pool(name="ipool", bufs=1) as ipool,
            tc.tile_pool(name="epool", bufs=20) as epool,
            tc.tile_pool(name="opool", bufs=4) as opool,
            tc.tile_pool(name="ps", bufs=8, space="PSUM") as pspool,
        ):

            def emit_xload(pair):
                """x DMA (hc-paired order) + x combo ops (DVE, per chunk)."""
                xb = xpool.tile([P, 8, TP], f16, tag="xb")
                xb_tiles[pair] = xb
                for k, hc in enumerate([0, 4, 1, 5, 2, 6, 3, 7]):
                    dma = nc.sync.dma_start(xb[:, hc, :], xp[pair, :, hc, :])
                    if pair > 0:
                        gated.append((dma, ("mm", 0, 5 + k)))
                xc = xcpool.tile([P, 5, KC, TH], f16, tag="xc")
                xc_tiles[pair] = xc
                for c in range(KC):
                    for slot, (h0, t0), (h1, t1), is_add in XC_SPEC:
                        nc.vector.tensor_tensor(
                            xc[:, slot, c, :],
                            xb[:, h0 + c, t0:t0 + TH],
                            xb[:, h1 + c, t1:t1 + TH],
                            ADD if is_add else SUB,
                        )

            def emit_g1(pair):
                xb, xc = xb_tiles[pair], xc_tiles[pair]
                it = ipool.tile([P, FO, TP], f16, tag="it")
                for j in range(NJ):
                    w1t = w1pool.tile([P, 7, KC, P], f16, tag="w1t")
                    if pair == 0 and j == 0:
                        # split: plain-A products' weights (i=2,3) first
                        nc.sync.dma_start(w1t[:, 2:4, :, :],
                                          w1c[0, :, 2 * KC * P:4 * KC * P])
                        nc.sync.dma_start(w1t[:, 0:2, :, :],
                                          w1c[0, :, 0:2 * KC * P])
                        nc.sync.dma_start(w1t[:, 4:7, :, :],
                                          w1c[0, :, 4 * KC * P:])
                    else:
                        d = nc.sync.dma_start(w1t[:], w1c[j])
                        if pair == 0 and j >= 2:
                            gated.append((d, ("mm", 0, j - 2)))
                        elif pair > 0:
                            gated.append((d, ("g2", 0, min(j, 15))))

                    # 7 Strassen product chains
                    psb = {}
                    for name, wi, rhs in CHAINS:
                        ps = pspool.tile([P, TH], f32, tag="ps")
                        psb[name] = ps
                        for c in range(KC):
                            if rhs[0] == "xb":
                                _, h0, t0 = rhs
                                r = xb[:, h0 + c, t0:t0 + TH]
                            else:
                                r = xc[:, rhs[1], c, :]
                            mm = nc.tensor.matmul(
                                ps[:], w1t[:, wi, c, :], r,
                                start=(c == 0), stop=(c == KC - 1),
                            )
                            if c == 0 and name == "p3":
                                mm_first[(pair, j)] = mm

                    # drain: recombine 7 products into the 4 C quadrants.
                    def et(nm):
                        return epool.tile([P, TH], f16, tag="e", name=nm)
                    e3, e4, e2, e5, e1 = (et("e3"), et("e4"), et("e2"),
                                          et("e5"), et("e1"))
                    nc.scalar.copy(e3[:], psb["p3"][:])
                    nc.vector.tensor_copy(e4[:], psb["p4"][:])
                    nc.vector.tensor_copy(e2[:], psb["p2"][:])
                    nc.scalar.copy(e5[:], psb["p5"][:])
                    nc.scalar.copy(e1[:], psb["p1"][:])
                    c21, c12, c22, c11 = (et("c21"), et("c12"), et("c22"),
                                          et("c11"))
                    ii, jj, g, hh = et("ti"), et("tj"), et("tg"), et("th")
                    nc.gpsimd.tensor_tensor(c21[:], e2[:], e4[:], ADD)
                    nc.gpsimd.tensor_tensor(c12[:], e3[:], e5[:], ADD)
                    nc.gpsimd.tensor_tensor(ii[:], e1[:], e2[:], SUB)
                    nc.vector.tensor_tensor(jj[:], ii[:], e3[:], ADD)
                    nc.vector.tensor_tensor(c22[:], psb["p6"][:], jj[:], ADD)
                    nc.vector.tensor_tensor(g[:], e1[:], e4[:], ADD)
                    nc.gpsimd.tensor_tensor(hh[:], g[:], e5[:], SUB)
                    nc.vector.tensor_tensor(c11[:], psb["p7"][:], hh[:], ADD)
                    # gelu -> inter quadrants (scalar)
                    nc.scalar.activation(it[:, j, TH:], c21[:], GELU)
                    nc.scalar.activation(it[:, j + NJ, 0:TH], c12[:], GELU)
                    nc.scalar.activation(it[:, j + NJ, TH:], c22[:], GELU)
                    nc.scalar.activation(it[:, j, 0:TH], c11[:], GELU)
                return it

            def emit_g2(pair, it):
                for ho in range(NHO):
                    w2t = w2pool.tile([P, F], f16, tag="w2t")
                    d = nc.sync.dma_start(w2t[:], w2p[ho])
                    if ho < 3:
                        gated.append((d, ("mm", pair, 9 + 2 * ho)))
                    else:
                        gated.append((d, ("g2", pair, 2 * (ho - 3))))
                    for th in range(2):
                        k = ho * 2 + th
                        ps = pspool.tile([P, TH], f32, tag="ps")
                        for fo in range(FO):
                            mm = nc.tensor.matmul(
                                ps[:],
                                w2t[:, fo * P:(fo + 1) * P],
                                it[:, fo, th * TH:(th + 1) * TH],
                                start=(fo == 0), stop=(fo == FO - 1),
                            )
                            if fo == 0:
                                g2_first[(pair, k)] = mm
                        ob = opool.tile([P, TH], f16, tag="ob")
                        if pair == NPAIR - 1 and k == 15:
                            # split eviction: first store overlaps second
                            # copy (shortens the kernel tail)
                            HB = TH // 2
                            nc.vector.tensor_copy(ob[:, :HB], ps[:, :HB])
                            nc.sync.dma_start(
                                out4[pair, th, ho, :, :HB], ob[:, :HB])
                            nc.vector.tensor_copy(ob[:, HB:], ps[:, HB:])
                            nc.sync.dma_start(
                                out4[pair, th, ho, :, HB:], ob[:, HB:])
                        else:
                            nc.vector.tensor_copy(ob[:], ps[:])
                            nc.sync.dma_start(out4[pair, th, ho], ob[:])

            # PE warm-up (HAM clock ramp) while the first operands stream in.
            warm = wpool.tile([P, TH], f16, tag="warm")
            nc.any.memset(warm[:], 0.0)
            for _ in range(NWARM):
                wp = pspool.tile([P, TH], f32, tag="ps")
                nc.tensor.matmul(wp[:], warm[:, 0:P], warm[:], start=True,
                                 stop=True)

            emit_xload(0)
            it0 = emit_g1(0)
            emit_xload(1)       # xb(1) DMAs gated; xc(1) DVE ops run during G2(0)
            emit_g2(0, it0)
            it1 = emit_g1(1)
            emit_g2(1, it1)

            # resolve DMA gates (stage weight stream behind compute)
            for dma, key in gated:
                tgt = (mm_first if key[0] == "mm" else g2_first)[
                    (key[1], key[2])]
                add_dep_helper(dma.ins, tgt.ins, sync=True,
                               reason="stage load behind compute")
    nc.compile()
    return nc


def _get_nc():
    global _nc_cache
    if _nc_cache is None:
        _nc_cache = _build_nc()
    return _nc_cache


def kernel(x, w1, w2, rows_per_expert):
    global LAST_RESULTS
    from concourse.bass_utils import run_bass_kernel_spmd

    x = np.asarray(x)
    w1 = np.asarray(w1)
    w2 = np.asarray(w2)
    rpe = int(rows_per_expert)
    assert x.shape == (E * rpe, H) and rpe == T_PER_E
    assert w1.shape == (E, H, F) and w2.shape == (E, F, H)

    f16 = np.float16
    in_maps = []
    for e in range(E):
        xe = x[e * rpe:(e + 1) * rpe].astype(f16)       # [T, H]
        # x[pr*1024+ti, hc*128+p] -> [pr, p, hc, ti]
        xpm = np.ascontiguousarray(
            xe.reshape(NPAIR, TP, 8, P).transpose(0, 3, 2, 1))
        # w1 Strassen block combos (fp32 on host, then fp16)
        w = w1[e]
        b11, b12 = w[:512, :2048], w[:512, 2048:]
        b21, b22 = w[512:, :2048], w[512:, 2048:]
        combos = np.stack([
            b11 + b22, b11, b12 - b22, b21 - b11, b22, b11 + b12, b21 + b22,
        ]).astype(f16)                                  # [7, 512, 2048]
        # w1c[j, p, i, c, fi] = combo[i, c*128+p, j*128+fi]
        w1m = np.ascontiguousarray(
            combos.reshape(7, KC, P, NJ, P).transpose(3, 2, 0, 1, 4)
        ).reshape(NJ, P, 7 * KC * P)
        # w2[f*128+p, h*128+hi] -> [h, p, f*128+hi]
        w2m = np.ascontiguousarray(
            w2[e].astype(f16).reshape(FO, P, NHO, P).transpose(2, 1, 0, 3)
        ).reshape(NHO, P, F)
        in_maps.append({"xp": xpm, "w1c": w1m, "w2p": w2m})

    res = run_bass_kernel_spmd(_get_nc(), in_maps, list(range(E)), trace=TRACE)
    LAST_RESULTS = res

    out = np.empty((E * rpe, H), dtype=np.float32)
    for e in range(E):
        # out4[pr, th, hc, p, ti] -> out[pr*1024+th*512+ti, hc*128+p]
        o4 = res.results[e]["out4"].astype(np.float32)
        out[e * rpe:(e + 1) * rpe] = (
            o4.transpose(0, 1, 4, 2, 3).reshape(rpe, H))
    return out


# revision 14
# speedup vs baseline: 1.4158x; 1.4158x over previous
"""Expert-parallel grouped MLP (MoE routing) for Trainium2,
bf16 + one-level Strassen on GEMM1.

Problem: x[16384,1024] fp32, w1[8,1024,4096], w2[8,4096,1024],
rows_per_expert=2048.  out = gelu(x_e @ w1[e]) @ w2[e] per expert group.

Sharding: one expert per NeuronCore (E=8 == n_cores), no collectives.

Speed strategy vs the bf16 direct baseline (466us -> ~439us):
  * One level of Strassen on GEMM1: per 1024-token pair, split
    [1024t,1024h]@[1024h,4096f] into 2x2x2 blocks and compute 7 products
    (4-chunk PSUM chains of 512-free matmuls) instead of 8 -> 12.5% fewer
    GEMM1 matmuls (1920 total vs 2048 direct).  The w1 block combos
    (B11+B22 etc.) are precomputed on the host in fp32; the x block
    combos are 20 cheap bf16 tensor_tensor ops on DVE.
  * Operands stay bf16: fp16 runs at the same cycles/row on the PE but
    its wider multiplier datapath trips the chip's HAM activity clamp
    (13/16 PE duty = +20% pitch) when all 8 cores stream random data -
    measured, not modeled.  Precision comes from fp32 drain/combines
    instead: rel-err 6.2e-3 vs gates rel<2e-2, resid_var<1e-4.
  * Product recombination (C11=P1+P4-P5+P7 etc.) uses the minimum 7
    PSUM-reading ops: 3 scalar evict-casts (P1,P2,P5 -> fp32 SBUF) and
    6 one-PSUM-operand DVE tensor_tensors, plus 2 GpSimd fp32 subs.
    The two 4-term quadrants chain through fp32 temps (a=P1-P5,
    b=P4+a, C11=P7+b; d=P1-P2, f=P3+d, C22=P6+f).
  * The drain is software-pipelined one group deep: late ops (P6/P7
    reads, gelus) are emitted after the NEXT group's casts so they never
    head-of-line block the in-order engine queues (this was worth 35us).
  * Quadrant pairs [C11|C21] land in one contiguous [128,1024] bf16 tile
    so gelu runs as 2 wide ops/group instead of 4.
  * Chain order p4,p2,p5,p1,p3,p6,p7 puts the late-read banks (p3,p6,p7)
    at the end of each group so next-group chains never wait on drains.
  * GEMM2 stays direct (its Strassen would need 5 inter-combo tensors of
    2MB each in SBUF - doesn't fit).
  * w1 combos (28MB over 2 pairs), w2 (16MB) and x stream behind compute
    via DMA gates on matmul anchors; first-group operand triggers are
    interleaved so the first real matmul issues ~13us after kernel start.

Layouts (host pre-permuted so every DMA line is >=1KB contiguous):
    xp  -> [2, 128, 8, 1024]   xp[pr,p,hc,ti]    = x[pr*1024+ti, hc*128+p]
    w1c -> [16, 128, 7*4*128]  w1c[j,p,i,c,fi]   = w1combo_i[c*128+p, j*128+fi]
    w2p -> [8, 128, 4096]      w2p[h,p,f*128+hi] = w2[f*128+p, h*128+hi]
    out4<- [2, 2, 8, 128, 512] out4[pr,th,hc,p,ti] = out[pr*1024+th*512+ti, hc*128+p]
"""

import numpy as np

E = 8
H = 1024
F = 4096
T_PER_E = 2048
P = 128
NPAIR = 2      # token pairs of 1024
TP = 1024      # tokens per pair
TH = 512       # free dim = half pair (Strassen M/2)
NJ = 16        # G1 f-tile groups per N-half
KC = 4         # G1 contraction chunks per product (512/128)
FO = 32        # f chunks of F
NHO = 8        # G2 h-tiles
NWARM = 12     # PE warm-up matmuls

# G1 Strassen chains, in PE emission order.
# (name, w1c product idx, rhs spec); rhs: ("xb", hc0, t0) = plain x block
# view, ("xc", slot) = x combo slot.
# Products (std Strassen): P1=(A11+A22)(B11+B22) P2=(A21+A22)B11
# P3=A11(B12-B22) P4=A22(B21-B11) P5=(A11+A12)B22 P6=(A21-A11)(B11+B12)
# P7=(A12-A22)(B21+B22)
# C11=P1+P4-P5+P7  C12=P3+P5  C21=P2+P4  C22=P1-P2+P3+P6
CHAINS = [
    ("p4", 3, ("xb", 4, TH)),
    ("p2", 1, ("xc", 1)),
    ("p5", 4, ("xc", 2)),
    ("p1", 0, ("xc", 0)),
    ("p3", 2, ("xb", 0, 0)),
    ("p6", 5, ("xc", 3)),
    ("p7", 6, ("xc", 4)),
]
# x combo slots: 0:A11+A22 1:A21+A22 2:A11+A12 3:A21-A11 4:A12-A22
# A11 = xb[:, c, 0:TH]   A12 = xb[:, 4+c, 0:TH]
# A21 = xb[:, c, TH:]    A22 = xb[:, 4+c, TH:]
XC_SPEC = [
    (0, (0, 0), (4, TH), True),
    (1, (0, TH), (4, TH), True),
    (2, (0, 0), (4, 0), True),
    (3, (0, TH), (0, 0), False),
    (4, (4, 0), (4, TH), False),
]

TRACE = False          # test.py sets kernel.TRACE = True for profiling
LAST_RESULTS = None    # BassKernelResults of the most recent run

_nc_cache = None


def _build_nc():
    import concourse.mybir as mybir
    import concourse.tile as tile
    from concourse import bacc
    from concourse.tile_rust import add_dep_helper

    f16 = mybir.dt.bfloat16
    f32 = mybir.dt.float32
    GELU = mybir.ActivationFunctionType.Gelu_apprx_tanh
    ADD = mybir.AluOpType.add
    SUB = mybir.AluOpType.subtract

    nc = bacc.Bacc("TRN2", target_bir_lowering=False, debug=False)

    xp = nc.dram_tensor("xp", [NPAIR, P, 8, TP], f16, kind="ExternalInput").ap()
    w1c = nc.dram_tensor("w1c", [NJ, P, 7 * KC * P], f16, kind="ExternalInput").ap()
    w2p = nc.dram_tensor("w2p", [NHO, P, F], f16, kind="ExternalInput").ap()
    out4 = nc.dram_tensor("out4", [NPAIR, 2, NHO, P, TH], f16,
                          kind="ExternalOutput").ap()

    mm_first = {}   # (pair, j) -> first matmul of G1 group
    g2_first = {}   # (pair, chain_k) -> first matmul of G2 chain
    gated = []      # (dma_instr, gate_key) resolved at the end
    xb_tiles = {}
    xc_tiles = {}
    w1t_first = {}

    with tile.TileContext(nc) as tc:
        with (
            tc.tile_pool(name="wpool", bufs=1) as wpool,
            tc.tile_pool(name="w1pool", bufs=3) as w1pool,
            tc.tile_pool(name="w2pool", bufs=3) as w2pool,
            tc.tile_pool(name="xpool", bufs=2) as xpool,
            tc.tile_pool(name="xcpool", bufs=1) as xcpool,
            tc.tile_pool(name="ipool", bufs=1) as ipool,
            tc.tile_pool(name="epool", bufs=10) as epool,
            tc.tile_pool(name="ccpool", bufs=4) as ccpool,
            tc.tile_pool(name="opool", bufs=4) as opool,
            tc.tile_pool(name="ps", bufs=8, space="PSUM") as pspool,
        ):

            def emit_xload(pair):
                """x DMA (hc-paired order) + x combo ops (DVE, per chunk).
                For pair 0 the first w1 chunk's triggers are interleaved so
                chain p4 (hc4 + w1c i=3) can start ~6us earlier."""
                xb = xpool.tile([P, 8, TP], f16, tag="xb")
                xc = xcpool.tile([P, 5, KC, TH], f16, tag="xc")
                xb_tiles[pair] = xb
                xc_tiles[pair] = xc
                if pair == 0:
                    # startup-critical trigger order: p4's operands (hc4-7 +
                    # w1c i<4) first; x combos are computed on DVE as the
                    # paired chunks land.
                    w1t0 = w1pool.tile([P, 7, KC, P], f16, tag="w1t")
                    w1t_first[0] = w1t0
                    nc.sync.dma_start(xb[:, 4, :], xp[0, :, 4, :])
                    nc.sync.dma_start(w1t0[:, 0:4, :, :],
                                      w1c[0, :, 0:4 * KC * P])
                    for hc in (5, 0, 6, 1):
                        nc.sync.dma_start(xb[:, hc, :], xp[0, :, hc, :])
                    nc.sync.dma_start(w1t0[:, 4:7, :, :],
                                      w1c[0, :, 4 * KC * P:])
                    for hc in (7, 2, 3):
                        nc.sync.dma_start(xb[:, hc, :], xp[0, :, hc, :])
                else:
                    for k, hc in enumerate([4, 0, 5, 1, 6, 2, 7, 3]):
                        dma = nc.sync.dma_start(xb[:, hc, :],
                                                xp[pair, :, hc, :])
                        gated.append((dma, ("g2", 0, min(15, 2 + k))))
                for c in range(KC):
                    for slot, (h0, t0), (h1, t1), is_add in XC_SPEC:
                        nc.vector.tensor_tensor(
                            xc[:, slot, c, :],
                            xb[:, h0 + c, t0:t0 + TH],
                            xb[:, h1 + c, t1:t1 + TH],
                            ADD if is_add else SUB,
                        )

            def emit_g1(pair):
                xb, xc = xb_tiles[pair], xc_tiles[pair]
                it = ipool.tile([P, FO, TP], f16, tag="it")
                pend = None

                def emit_late(st):
                    pj, psb, cc1, cc2, b, dd, ff = st
                    # C11 = P7 + (P1+P4-P5); C22 = P6 + (P1-P2+P3)
                    nc.vector.tensor_tensor(cc1[:, 0:TH], psb["p7"][:], b[:],
                                            ADD)
                    nc.vector.tensor_tensor(ff[:], psb["p3"][:], dd[:], ADD)
                    nc.vector.tensor_tensor(cc2[:, TH:], psb["p6"][:], ff[:],
                                            ADD)
                    nc.scalar.activation(it[:, pj, :], cc1[:], GELU)
                    nc.scalar.activation(it[:, pj + NJ, :], cc2[:], GELU)

                for j in range(NJ):
                    if pair == 0 and j == 0:
                        w1t = w1t_first[0]   # DMA'd during emit_xload(0)
                    else:
                        w1t = w1pool.tile([P, 7, KC, P], f16, tag="w1t")
                        d = nc.sync.dma_start(w1t[:], w1c[j])
                        if pair == 0 and j >= 2:
                            gated.append((d, ("mm", 0, j - 2)))
                        elif pair > 0 and j < 4:
                            gated.append((d, ("g2", 0, 11 + j)))
                        elif pair > 0:
                            gated.append((d, ("mm", 1, j - 2)))

                    # 7 Strassen product chains
                    psb = {}
                    for name, wi, rhs in CHAINS:
                        ps = pspool.tile([P, TH], f32, tag="ps")
                        psb[name] = ps
                        for c in range(KC):
                            if rhs[0] == "xb":
                                _, h0, t0 = rhs
                                r = xb[:, h0 + c, t0:t0 + TH]
                            else:
                                r = xc[:, rhs[1], c, :]
                            mm = nc.tensor.matmul(
                                ps[:], w1t[:, wi, c, :], r,
                                start=(c == 0), stop=(c == KC - 1),
                            )
                            if c == 0 and name == "p4":
                                mm_first[(pair, j)] = mm

                    # drain: recombine the 7 products into the 4 C
                    # quadrants, software-pipelined one group deep so the
                    # late ops (P6/P7 reads + gelus) never head-of-line
                    # block the next group's casts in the in-order queues.
                    def et(nm):
                        return epool.tile([P, TH], f32, tag="e", name=nm)
                    e2, e5, e1 = et("e2"), et("e5"), et("e1")
                    nc.scalar.copy(e2[:], psb["p2"][:])
                    nc.scalar.copy(e5[:], psb["p5"][:])
                    nc.scalar.copy(e1[:], psb["p1"][:])
                    if pend is not None:
                        emit_late(pend)
                    # cc1 = [C11 | C21] -> it[:, j, :]; cc2 = [C12 | C22]
                    cc1 = ccpool.tile([P, TP], f16, tag="cc", name="cc1")
                    cc2 = ccpool.tile([P, TP], f16, tag="cc", name="cc2")
                    nc.vector.tensor_tensor(cc1[:, TH:], psb["p4"][:], e2[:],
                                            ADD)             # C21 = P4+P2
                    nc.vector.tensor_tensor(cc2[:, 0:TH], psb["p3"][:], e5[:],
                                            ADD)             # C12 = P3+P5
                    a, dd, b, ff = et("a"), et("dd"), et("b"), et("ff")
                    nc.gpsimd.tensor_tensor(a[:], e1[:], e5[:], SUB)
                    nc.gpsimd.tensor_tensor(dd[:], e1[:], e2[:], SUB)
                    nc.vector.tensor_tensor(b[:], psb["p4"][:], a[:], ADD)
                    pend = (j, psb, cc1, cc2, b, dd, ff)

                emit_late(pend)
                return it

            def emit_g2(pair, it):
                for ho in range(NHO):
                    w2t = w2pool.tile([P, F], f16, tag="w2t")
                    d = nc.sync.dma_start(w2t[:], w2p[ho])
                    if ho < 3:
                        gated.append((d, ("mm", pair, 9 + 2 * ho)))
                    else:
                        gated.append((d, ("g2", pair, 2 * (ho - 3))))
                    for th in range(2):
                        k = ho * 2 + th
                        ps = pspool.tile([P, TH], f32, tag="ps")
                        for fo in range(FO):
                            mm = nc.tensor.matmul(
                                ps[:],
                                w2t[:, fo * P:(fo + 1) * P],
                                it[:, fo, th * TH:(th + 1) * TH],
                                start=(fo == 0), stop=(fo == FO - 1),
                            )
                            if fo == 0:
                                g2_first[(pair, k)] = mm
                        ob = opool.tile([P, TH], f16, tag="ob")
                        if pair == NPAIR - 1 and k == 15:
                            # final eviction: halves copied on DVE and
                            # Scalar in parallel (shortens the kernel tail)
                            HB = TH // 2
                            nc.vector.tensor_copy(ob[:, :HB], ps[:, :HB])
                            nc.scalar.copy(ob[:, HB:], ps[:, HB:])
                            nc.sync.dma_start(
                                out4[pair, th, ho, :, :HB], ob[:, :HB])
                            nc.sync.dma_start(
                                out4[pair, th, ho, :, HB:], ob[:, HB:])
                        else:
                            nc.vector.tensor_copy(ob[:], ps[:])
                            nc.sync.dma_start(out4[pair, th, ho], ob[:])

            # PE warm-up (HAM clock ramp) while the first operands stream in.
            warm = wpool.tile([P, TH], f16, tag="warm")
            nc.any.memset(warm[:], 0.0)
            for wk in range(NWARM):
                # graduated widths ramp PE activity (and the HAM power
                # estimate) gradually instead of a step surge
                wid = [64, 64, 128, 128, 256, 256, 256, 512, 512, 512,
                       512, 512][wk % 12]
                wp = pspool.tile([P, TH], f32, tag="ps")
                nc.tensor.matmul(wp[:, 0:wid], warm[:, 0:P], warm[:, 0:wid],
                                 start=True, stop=True)

            emit_xload(0)
            it0 = emit_g1(0)
            emit_xload(1)       # xb(1) DMAs gated; xc(1) DVE ops run during G2(0)
            emit_g2(0, it0)
            it1 = emit_g1(1)
            emit_g2(1, it1)

            # resolve DMA gates (stage weight stream behind compute)
            for dma, key in gated:
                tgt = (mm_first if key[0] == "mm" else g2_first)[
                    (key[1], key[2])]
                add_dep_helper(dma.ins, tgt.ins, sync=True,
                               reason="stage load behind compute")
    nc.compile()
    return nc


def _get_nc():
    global _nc_cache
    if _nc_cache is None:
        _nc_cache = _build_nc()
    return _nc_cache


def kernel(x, w1, w2, rows_per_expert):
    global LAST_RESULTS
    from concourse.bass_utils import run_bass_kernel_spmd

    x = np.asarray(x)
    w1 = np.asarray(w1)
    w2 = np.asarray(w2)
    rpe = int(rows_per_expert)
    assert x.shape == (E * rpe, H) and rpe == T_PER_E
    assert w1.shape == (E, H, F) and w2.shape == (E, F, H)

    import ml_dtypes
    f16 = ml_dtypes.bfloat16
    in_maps = []
    for e in range(E):
        xe = x[e * rpe:(e + 1) * rpe].astype(f16)       # [T, H]
        # x[pr*1024+ti, hc*128+p] -> [pr, p, hc, ti]
        xpm = np.ascontiguousarray(
            xe.reshape(NPAIR, TP, 8, P).transpose(0, 3, 2, 1))
        # w1 Strassen block combos (fp32 on host, then fp16)
        w = w1[e]
        b11, b12 = w[:512, :2048], w[:512, 2048:]
        b21, b22 = w[512:, :2048], w[512:, 2048:]
        combos = np.stack([
            b11 + b22, b11, b12 - b22, b21 - b11, b22, b11 + b12, b21 + b22,
        ]).astype(f16)                                  # [7, 512, 2048]
        # w1c[j, p, i, c, fi] = combo[i, c*128+p, j*128+fi]
        w1m = np.ascontiguousarray(
            combos.reshape(7, KC, P, NJ, P).transpose(3, 2, 0, 1, 4)
        ).reshape(NJ, P, 7 * KC * P)
        # w2[f*128+p, h*128+hi] -> [h, p, f*128+hi]
        w2m = np.ascontiguousarray(
            w2[e].astype(f16).reshape(FO, P, NHO, P).transpose(2, 1, 0, 3)
        ).reshape(NHO, P, F)
        in_maps.append({"xp": xpm, "w1c": w1m, "w2p": w2m})

    res = run_bass_kernel_spmd(_get_nc(), in_maps, list(range(E)), trace=TRACE)
    LAST_RESULTS = res

    out = np.empty((E * rpe, H), dtype=np.float32)
    for e in range(E):
        # out4[pr, th, hc, p, ti] -> out[pr*1024+th*512+ti, hc*128+p]
        o4 = res.results[e]["out4"].astype(np.float32)
        out[e * rpe:(e + 1) * rpe] = (
            o4.transpose(0, 1, 4, 2, 3).reshape(rpe, H))
    return out
